# revision 3
# baseline (speedup 1.0000x reference)
"""Trainium2 Bass kernel for ConvolutionalAttention2D (linear attention with 1x1 convs).

Reference computation (per batch b):
    q = Wq x ; k = Wk x ; v = Wv x          (1x1 convs == channel matmuls)
    phi(t) = elu(t) + 1
    qv = phi(q) @ phi(v)^T                  ([C, C] context matrix, contract over pixels)
    out = Wo (qv @ phi(k)) + bo

Kernel strategy (8 NeuronCores, data-parallel over batch B=16 -> 2 batches/core):
  - All PE traffic in bf16 (x converted on host); fp32 PSUM accumulate.
  - Algebraic refactor: Wo (qv @ phi_k) == (Wo qv) @ phi_k.
  - phi(t) = min(exp(t), 1) + relu(t), split across ACT/DVE/GPSIMD per-span
    to balance engine load (ACT does only the mandatory Exp pass by default).
  - Out-copies (PSUM->SBUF + bias) placed mostly on ACT slack.
"""

from contextlib import ExitStack

import numpy as np

import concourse.bacc as bacc
import concourse.tile as tile
from concourse import mybir
from concourse import bass_utils

B, C, H, W = 16, 256, 64, 64
HW = H * W
NCORES = 8
NB = B // NCORES  # batches per core

FP = mybir.dt.float32
BF = mybir.dt.bfloat16
AF = mybir.ActivationFunctionType
OP = mybir.AluOpType

# per-batch span schemes: 24 spans (16 stage-B + 8 stage-A)
#  'p': ACT Exp ; GPSIMD min(e,1) ; DVE stt(psum max 0 + t)   [default]
#  'a': ACT Exp ; DVE min(e,1) 4x ; DVE stt(psum max 0 + t)
#  'b': ACT Exp ; ACT Relu ; DVE stt(e min 1 + r) all-bf16
DEFAULT_SPANS = "p" * 24
# per-batch out-copy engine for the 8 stage-D tiles: 'A' = ACT w/ bias, 'V' = DVE
DEFAULT_OUTS = "AAAAAAAV"


def flat2(ap):
    return ap.rearrange("p a b -> p (a b)")


def build_kernel(repeat: int = 1, span_pattern=DEFAULT_SPANS, out_pattern=DEFAULT_OUTS,
                 xp_bufs=4, phikp_bufs=2, pqvp_bufs=2, mm_bufs=3, tmps_bufs=6,
                 outp_bufs=6, out_fp32=True):
    """Build the per-core Bass program. `repeat` wraps the whole body in a
    dynamic For_i loop (used only for wall-clock timing runs)."""
    nc = bacc.Bacc("TRN2", target_bir_lowering=False, debug=False)

    x_d = nc.dram_tensor("x", [NB, 2, 128, HW], BF, kind="ExternalInput")
    # weights packed per cc-chunk: [wqv(512) | wk(256) | wo(256)]
    w_d = nc.dram_tensor("wall", [2, 128, 1024], BF, kind="ExternalInput")
    bo_d = nc.dram_tensor("bo", [C, 1], FP, kind="ExternalInput")
    OD = FP if out_fp32 else BF
    out_d = nc.dram_tensor("out", [NB, C, HW], OD, kind="ExternalOutput")

    with tile.TileContext(nc) as tc, ExitStack() as ctx:
        singles = ctx.enter_context(tc.tile_pool(name="singles", bufs=1))
        xp = ctx.enter_context(tc.tile_pool(name="xp", bufs=xp_bufs))
        phikp = ctx.enter_context(tc.tile_pool(name="phikp", bufs=phikp_bufs))
        pqvp = ctx.enter_context(tc.tile_pool(name="pqvp", bufs=pqvp_bufs))
        tmps = ctx.enter_context(tc.tile_pool(name="tmps", bufs=tmps_bufs))
        smalls = ctx.enter_context(tc.tile_pool(name="smalls", bufs=2))
        outp = ctx.enter_context(tc.tile_pool(name="outp", bufs=outp_bufs))
        psmm = ctx.enter_context(tc.tile_pool(name="psmm", bufs=mm_bufs, space="PSUM"))
        psacc = ctx.enter_context(tc.tile_pool(name="psacc", bufs=1, space="PSUM"))

        # ---- weights (loaded once, replicated) ----
        w_all = singles.tile([128, 2, 1024], BF, tag="wall")
        nc.sync.dma_start(out=w_all[:], in_=w_d.ap().rearrange("cc p b -> p cc b"))
        wqv = [w_all[:, cc, 0:512] for cc in range(2)]
        wk = [w_all[:, cc, 512:768] for cc in range(2)]
        wo = [w_all[:, cc, 768:1024] for cc in range(2)]
        bo_sb = singles.tile([128, 2], FP, tag="bo")
        for m in range(2):
            nc.sync.dma_start(
                out=bo_sb[:, m:m + 1], in_=bo_d.ap()[m * 128:(m + 1) * 128, :]
            )

        state = {"span": 0, "out": 0}

        def phi_span(psum_ap, dst_ap):
            """dst = phi(psum) = min(exp(x), 1) + relu(x), bf16 out."""
            i = state["span"] % 24
            state["span"] += 1
            sch = span_pattern[i]
            e = tmps.tile([128, 1024], BF, tag="e")
            nc.scalar.activation(e[:], psum_ap, AF.Exp)
            if sch == "b":
                r = tmps.tile([128, 1024], BF, tag="r")
                nc.scalar.activation(r[:], psum_ap, AF.Relu)
                nc.vector.scalar_tensor_tensor(dst_ap, e[:], 1.0, r[:], OP.min, OP.add)
            else:
                t = tmps.tile([128, 1024], BF, tag="t")
                if sch == "p":
                    nc.gpsimd.tensor_scalar_min(t[:], e[:], 1.0)
                else:
                    nc.vector.tensor_scalar_min(t[:], e[:], 1.0)
                nc.vector.scalar_tensor_tensor(dst_ap, psum_ap, 0.0, t[:], OP.max, OP.add)

        def body(_iv=None):
            state["span"] = 0
            state["out"] = 0
            for b in range(NB):
                # ---- load x for this batch in column blocks ----
                X = [xp.tile([128, HW], BF, tag="x", name=f"x{b}_{cc}") for cc in range(2)]
                xblocks = [(0, 512), (512, 512), (1024, 1024), (2048, 1024), (3072, 1024)]
                for (c0, cw) in xblocks:
                    cs = slice(c0, c0 + cw)
                    for cc in range(2):
                        nc.sync.dma_start(
                            out=X[cc][:, cs],
                            in_=x_d.ap()[b, cc, :, cs],
                        )

                # ---- stage B: phi(q^T), phi(v^T), transposed layout [n, o] ----
                # pqv_sb[:, nchunk, 0:256] = phi_qT, [:, nchunk, 256:512] = phi_vT
                pqv_sb = pqvp.tile([128, 32, 512], BF, tag="pqv")
                for i in range(16):
                    ps = psmm.tile([128, 2, 512], FP, tag="mm")
                    for j in range(2):
                        nk = i * 2 + j
                        for cc in range(2):
                            nc.tensor.matmul(
                                ps[:, j, :],
                                X[cc][:, nk * 128:(nk + 1) * 128],
                                wqv[cc],
                                start=(cc == 0),
                                stop=(cc == 1),
                            )
                    phi_span(flat2(ps[:]), flat2(pqv_sb[:, i * 2:(i + 1) * 2, :]))

                # ---- stage A: phi_k = phi(Wk x), natural layout [o, n] ----
                phik = []
                for m in range(2):
                    pk = phikp.tile([128, HW], BF, tag="phik")
                    phik.append(pk)
                    for i in range(4):
                        ps = psmm.tile([128, 1024], FP, tag="mm")
                        for j in range(2):
                            n0 = (i * 2 + j) * 512
                            for cc in range(2):
                                nc.tensor.matmul(
                                    ps[:, j * 512:(j + 1) * 512],
                                    wk[cc][:, m * 128:(m + 1) * 128],
                                    X[cc][:, n0:n0 + 512],
                                    start=(cc == 0),
                                    stop=(cc == 1),
                                )
                        phi_span(ps[:], pk[:, i * 1024:(i + 1) * 1024])

                # ---- stage C: qv[c, d] = sum_n phi_qT[n, c] phi_vT[n, d] ----
                # interleaved cc accumulation chains -> each in its own bank
                qv_ps = psacc.tile([128, 2, 512], FP, tag="acc")
                for i in range(32):
                    for cc in range(2):
                        nc.tensor.matmul(
                            qv_ps[:, cc, 0:256],
                            pqv_sb[:, i, cc * 128:(cc + 1) * 128],
                            pqv_sb[:, i, 256:512],
                            start=(i == 0),
                            stop=(i == 31),
                        )
                qv_sb = smalls.tile([128, 2, 256], BF, tag="qv_sb")
                nc.scalar.activation(qv_sb[:], qv_ps[:, :, 0:256], AF.Copy)

                # ---- stage C2: W2^T[d, o] = sum_c qv[c, d] WoT[c, o] ----
                w2_ps = psacc.tile([128, 2, 256], FP, tag="acc")
                for dd in range(2):
                    for cc in range(2):
                        nc.tensor.matmul(
                            w2_ps[:, dd, :],
                            qv_sb[:, cc, dd * 128:(dd + 1) * 128],
                            wo[cc],
                            start=(cc == 0),
                            stop=(cc == 1),
                        )
                w2_sb = smalls.tile([128, 2, 256], BF, tag="w2_sb")
                nc.scalar.activation(flat2(w2_sb[:]), flat2(w2_ps[:]), AF.Copy)

                # ---- stage D: out[o, n] = sum_d W2[o, d] phi_k[d, n] + bo ----
                for m in range(2):
                    for i in range(4):
                        ps = psmm.tile([128, 1024], FP, tag="mm")
                        for j in range(2):
                            n0 = (i * 2 + j) * 512
                            for dd in range(2):
                                nc.tensor.matmul(
                                    ps[:, j * 512:(j + 1) * 512],
                                    w2_sb[:, dd, m * 128:(m + 1) * 128],
                                    phik[dd][:, n0:n0 + 512],
                                    start=(dd == 0),
                                    stop=(dd == 1),
                                )
                        o_sb = outp.tile([128, 1024], OD, tag="osb")
                        oe = out_pattern[state["out"] % 8]
                        if oe == "A":
                            nc.scalar.activation(
                                o_sb[:], ps[:], AF.Identity, bias=bo_sb[:, m:m + 1]
                            )
                        else:
                            nc.vector.tensor_scalar_add(o_sb[:], ps[:], bo_sb[:, m:m + 1])
                        state["out"] += 1
                        nc.sync.dma_start(
                            out=out_d.ap()[b, m * 128:(m + 1) * 128, i * 1024:(i + 1) * 1024],
                            in_=o_sb[:],
                        )

        if repeat == 1:
            body()
        else:
            with tc.For_i(0, repeat, 1) as iv:
                body(iv)

    nc.compile()
    return nc


_nc_cache = {}


def _get_nc(repeat: int = 1):
    if repeat not in _nc_cache:
        _nc_cache[repeat] = build_kernel(repeat)
    return _nc_cache[repeat]


def _to_bf16(a):
    import ml_dtypes
    return np.asarray(a, dtype=np.float32).astype(ml_dtypes.bfloat16)


def make_in_maps(x, Wq, Wk, Wv, Wo, bo):
    x = np.asarray(x, dtype=np.float32).reshape(B, 2, 128, HW)
    xb = np.ascontiguousarray(_to_bf16(x))
    # pack weights: per cc chunk rows [128], cols [wq(256)|wv(256)|wk(256)|wo(256)]
    wq_t = np.asarray(Wq, dtype=np.float32).T.reshape(2, 128, C)
    wv_t = np.asarray(Wv, dtype=np.float32).T.reshape(2, 128, C)
    wk_t = np.asarray(Wk, dtype=np.float32).T.reshape(2, 128, C)
    wo_t = np.asarray(Wo, dtype=np.float32).T.reshape(2, 128, C)
    wall = np.concatenate([wq_t, wv_t, wk_t, wo_t], axis=2)  # [2, 128, 1024]
    wall = np.ascontiguousarray(_to_bf16(wall))
    bo2 = np.ascontiguousarray(np.asarray(bo, dtype=np.float32).reshape(C, 1))
    return [
        {"x": xb[i * NB:(i + 1) * NB], "wall": wall, "bo": bo2}
        for i in range(NCORES)
    ]


def kernel(x, Wq, Wk, Wv, Wo, bo):
    nc = _get_nc(repeat=1)
    in_maps = make_in_maps(x, Wq, Wk, Wv, Wo, bo)
    res = bass_utils.run_bass_kernel_spmd(nc, in_maps, core_ids=list(range(NCORES)))
    out = np.concatenate([res.results[i]["out"] for i in range(NCORES)], axis=0)
    return np.ascontiguousarray(out.reshape(B, C, H, W).astype(np.float32))


# revision 8
# speedup vs baseline: 3.5136x; 3.5136x over previous
"""Trainium2 Bass kernel for ConvolutionalAttention2D (linear attention with 1x1 convs).

Reference computation (per batch b):
    q = Wq x ; k = Wk x ; v = Wv x          (1x1 convs == channel matmuls)
    phi(t) = elu(t) + 1
    qv = phi(q) @ phi(v)^T                  ([C, C] context matrix, contract over pixels)
    out = Wo (qv @ phi(k)) + bo

Kernel strategy (8 NeuronCores, data-parallel over batch B=16 -> 2 batches/core):
  - All PE traffic in bf16 (x converted on host); fp32 PSUM accumulate.
  - Algebraic refactor: Wo (qv @ phi_k) == (Wo qv) @ phi_k.
  - phi(t) = min(exp(t), 1) + relu(t), split across ACT/DVE/GPSIMD per-span
    to balance engine load (ACT does only the mandatory Exp pass by default).
  - Out-copies (PSUM->SBUF + bias) placed mostly on ACT slack.
"""

from contextlib import ExitStack

import numpy as np

import concourse.bacc as bacc
import concourse.tile as tile
from concourse import mybir
from concourse import bass_utils

B, C, H, W = 16, 256, 64, 64
HW = H * W
NCORES = 8
NB = B // NCORES  # batches per core

FP = mybir.dt.float32
BF = mybir.dt.bfloat16
AF = mybir.ActivationFunctionType
OP = mybir.AluOpType


def _register_phi_op():
    """Register a fused DVE op: out = max(min(in0, s0), in1 + s1).

    With in0 = exp(x) and s0 = s1 = 1 this is exactly phi(x) = elu(x)+1:
    for x <= 0, e^x >= 1+x so max picks e^x (and e^x <= 1); for x > 0,
    min clamps to 1 and max picks 1+x.
    """
    from concourse import dve_ops as D
    from concourse.dve_spec import Spec, Src0, Src1, C0, C1, maxx, minn, lower, _has_src1
    from concourse.dve_uop import DveOpSpec

    name = "PHI_COMBINE_ANT"
    for op in D.OPS:
        if op.name == name:
            return op
    spec = Spec(
        body=maxx(minn(Src0, C0), Src1 + C1),
        reference=lambda in0, in1, s0, s1, imm2: np.maximum(
            np.minimum(in0.astype(np.float32), s0), in1.astype(np.float32) + s1
        ),
    )
    shas = {}
    for ver in ("v3", "v4"):
        u = lower(spec, ver=ver)
        shas[ver] = DveOpSpec(
            name=name, opcode=0, uops=u, rd1_en=_has_src1(spec)
        ).sha(ver)
    op = D.DveOp(name, spec, subdim=False, uops_sha=shas)
    D.OPS.append(op)
    D._SUB_OPCODE_FOR_NAME[name] = D._CUSTOM_DVE_ROW_BASE + len(D.OPS) - 1
    D.CUSTOM_DVE_SPECS[name] = spec
    return op


PHI_OP = _register_phi_op()

# per-batch span schemes: 24 spans (16 stage-B + 8 stage-A)
#  'f': ACT Exp ; fused DVE max(min(e,1), psum+1)             [default]
#  'a': ACT Exp ; DVE min(e,1) 4x ; DVE stt(psum max 0 + t)
#  'b': ACT Exp ; ACT Relu ; DVE stt(e min 1 + r) all-bf16
DEFAULT_SPANS = "f" * 24
# per-batch out-copy engine for the 8 stage-D tiles: 'A' = ACT w/ bias, 'V' = DVE
DEFAULT_OUTS = "AAAAAAVV"


def flat2(ap):
    return ap.rearrange("p a b -> p (a b)")


def build_kernel(repeat: int = 1, span_pattern=DEFAULT_SPANS, out_pattern=DEFAULT_OUTS,
                 xp_bufs=4, phikp_bufs=2, pqvp_bufs=2, mm_bufs=3, tmps_bufs=6,
                 outp_bufs=6, out_fp32=True):
    """Build the per-core Bass program. `repeat` wraps the whole body in a
    dynamic For_i loop (used only for wall-clock timing runs)."""
    nc = bacc.Bacc("TRN2", target_bir_lowering=False, debug=False)

    x_d = nc.dram_tensor("x", [NB, 2, 128, HW], BF, kind="ExternalInput")
    # weights packed per cc-chunk: [wqv(512) | wk(256) | wo(256)]
    w_d = nc.dram_tensor("wall", [2, 128, 1024], BF, kind="ExternalInput")
    bo_d = nc.dram_tensor("bo", [C, 1], FP, kind="ExternalInput")
    OD = FP if out_fp32 else BF
    out_d = nc.dram_tensor("out", [NB, C, HW], OD, kind="ExternalOutput")

    with tile.TileContext(nc) as tc, ExitStack() as ctx:
        singles = ctx.enter_context(tc.tile_pool(name="singles", bufs=1))
        xp = ctx.enter_context(tc.tile_pool(name="xp", bufs=xp_bufs))
        phikp = ctx.enter_context(tc.tile_pool(name="phikp", bufs=phikp_bufs))
        pqvp = ctx.enter_context(tc.tile_pool(name="pqvp", bufs=pqvp_bufs))
        tmps = ctx.enter_context(tc.tile_pool(name="tmps", bufs=tmps_bufs))
        smalls = ctx.enter_context(tc.tile_pool(name="smalls", bufs=2))
        outp = ctx.enter_context(tc.tile_pool(name="outp", bufs=outp_bufs))
        psmm = ctx.enter_context(tc.tile_pool(name="psmm", bufs=mm_bufs, space="PSUM"))
        psacc = ctx.enter_context(tc.tile_pool(name="psacc", bufs=1, space="PSUM"))

        # ---- weights (loaded once, replicated) ----
        w_all = singles.tile([128, 2, 1024], BF, tag="wall")
        nc.sync.dma_start(out=w_all[:], in_=w_d.ap().rearrange("cc p b -> p cc b"))
        wqv = [w_all[:, cc, 0:512] for cc in range(2)]
        wk = [w_all[:, cc, 512:768] for cc in range(2)]
        wo = [w_all[:, cc, 768:1024] for cc in range(2)]
        bo_sb = singles.tile([128, 2], FP, tag="bo")
        for m in range(2):
            nc.sync.dma_start(
                out=bo_sb[:, m:m + 1], in_=bo_d.ap()[m * 128:(m + 1) * 128, :]
            )

        state = {"span": 0, "out": 0}

        def phi_span(psum3_ap, dst_ap):
            """dst = phi(psum) = max(min(exp(x), 1), x + 1), bf16 out.

            psum3_ap must be rank-3 [128, a, b] (the custom-DVE STT struct
            requires a 2-free-dim src1); dst_ap free size must match.
            """
            i = state["span"] % 24
            state["span"] += 1
            sch = span_pattern[i]
            flat = psum3_ap.rearrange("p a b -> p (a b)")
            e = tmps.tile([128, 1024], BF, tag="e")
            nc.scalar.activation(e[:], flat, AF.Exp)
            if sch == "f":
                nc.vector._custom_dve(
                    PHI_OP, out=dst_ap, in0=e[:], in1=psum3_ap, s0=1.0, s1=1.0
                )
            elif sch == "b":
                r = tmps.tile([128, 1024], BF, tag="r")
                nc.scalar.activation(r[:], flat, AF.Relu)
                nc.vector.scalar_tensor_tensor(dst_ap, e[:], 1.0, r[:], OP.min, OP.add)
            else:
                t = tmps.tile([128, 1024], BF, tag="t")
                nc.vector.tensor_scalar_min(t[:], e[:], 1.0)
                nc.vector.scalar_tensor_tensor(dst_ap, flat, 0.0, t[:], OP.max, OP.add)

        def body(_iv=None):
            state["span"] = 0
            state["out"] = 0
            for b in range(NB):
                # ---- load x for this batch in column blocks ----
                X = [xp.tile([128, HW], BF, tag="x", name=f"x{b}_{cc}") for cc in range(2)]
                xblocks = [(0, 512), (512, 512), (1024, 1024), (2048, 1024), (3072, 1024)]
                for (c0, cw) in xblocks:
                    cs = slice(c0, c0 + cw)
                    for cc in range(2):
                        nc.sync.dma_start(
                            out=X[cc][:, cs],
                            in_=x_d.ap()[b, cc, :, cs],
                        )

                # ---- stage B: phi(q^T), phi(v^T), transposed layout [n, o] ----
                # pqv_sb[:, nchunk, 0:256] = phi_qT, [:, nchunk, 256:512] = phi_vT
                pqv_sb = pqvp.tile([128, 32, 512], BF, tag="pqv")
                for i in range(16):
                    ps = psmm.tile([128, 2, 512], FP, tag="mm")
                    for j in range(2):
                        nk = i * 2 + j
                        for cc in range(2):
                            nc.tensor.matmul(
                                ps[:, j, :],
                                X[cc][:, nk * 128:(nk + 1) * 128],
                                wqv[cc],
                                start=(cc == 0),
                                stop=(cc == 1),
                            )
                    phi_span(ps[:], pqv_sb[:, i * 2:(i + 1) * 2, :])

                # ---- stage A: phi_k = phi(Wk x), natural layout [o, n] ----
                phik = []
                for m in range(2):
                    pk = phikp.tile([128, HW], BF, tag="phik")
                    phik.append(pk)
                    for i in range(4):
                        ps = psmm.tile([128, 2, 512], FP, tag="mm")
                        for j in range(2):
                            n0 = (i * 2 + j) * 512
                            for cc in range(2):
                                nc.tensor.matmul(
                                    ps[:, j, :],
                                    wk[cc][:, m * 128:(m + 1) * 128],
                                    X[cc][:, n0:n0 + 512],
                                    start=(cc == 0),
                                    stop=(cc == 1),
                                )
                        phi_span(ps[:], pk[:, i * 1024:(i + 1) * 1024])

                # ---- stage C: qv[c, d] = sum_n phi_qT[n, c] phi_vT[n, d] ----
                # interleaved cc accumulation chains -> each in its own bank
                qv_ps = psacc.tile([128, 2, 512], FP, tag="acc")
                for i in range(32):
                    for cc in range(2):
                        nc.tensor.matmul(
                            qv_ps[:, cc, 0:256],
                            pqv_sb[:, i, cc * 128:(cc + 1) * 128],
                            pqv_sb[:, i, 256:512],
                            start=(i == 0),
                            stop=(i == 31),
                        )
                qv_sb = smalls.tile([128, 2, 256], BF, tag="qv_sb")
                nc.scalar.activation(qv_sb[:], qv_ps[:, :, 0:256], AF.Copy)

                # ---- stage C2: W2^T[d, o] = sum_c qv[c, d] WoT[c, o] ----
                w2_ps = psacc.tile([128, 2, 256], FP, tag="acc")
                for dd in range(2):
                    for cc in range(2):
                        nc.tensor.matmul(
                            w2_ps[:, dd, :],
                            qv_sb[:, cc, dd * 128:(dd + 1) * 128],
                            wo[cc],
                            start=(cc == 0),
                            stop=(cc == 1),
                        )
                w2_sb = smalls.tile([128, 2, 256], BF, tag="w2_sb")
                nc.scalar.activation(flat2(w2_sb[:]), flat2(w2_ps[:]), AF.Copy)

                # ---- stage D: out[o, n] = sum_d W2[o, d] phi_k[d, n] + bo ----
                for m in range(2):
                    for i in range(4):
                        ps = psmm.tile([128, 1024], FP, tag="mm")
                        for j in range(2):
                            n0 = (i * 2 + j) * 512
                            for dd in range(2):
                                nc.tensor.matmul(
                                    ps[:, j * 512:(j + 1) * 512],
                                    w2_sb[:, dd, m * 128:(m + 1) * 128],
                                    phik[dd][:, n0:n0 + 512],
                                    start=(dd == 0),
                                    stop=(dd == 1),
                                )
                        o_sb = outp.tile([128, 1024], OD, tag="osb")
                        oe = out_pattern[state["out"] % 8]
                        if oe == "A":
                            nc.scalar.activation(
                                o_sb[:], ps[:], AF.Identity, bias=bo_sb[:, m:m + 1]
                            )
                        else:
                            nc.vector.tensor_scalar_add(o_sb[:], ps[:], bo_sb[:, m:m + 1])
                        state["out"] += 1
                        nc.sync.dma_start(
                            out=out_d.ap()[b, m * 128:(m + 1) * 128, i * 1024:(i + 1) * 1024],
                            in_=o_sb[:],
                        )

        if repeat == 1:
            body()
        else:
            with tc.For_i(0, repeat, 1) as iv:
                body(iv)

    nc.compile()
    return nc


_nc_cache = {}


def _get_nc(repeat: int = 1):
    if repeat not in _nc_cache:
        _nc_cache[repeat] = build_kernel(repeat)
    return _nc_cache[repeat]


def _to_bf16(a):
    import ml_dtypes
    return np.asarray(a, dtype=np.float32).astype(ml_dtypes.bfloat16)


def make_in_maps(x, Wq, Wk, Wv, Wo, bo):
    x = np.asarray(x, dtype=np.float32).reshape(B, 2, 128, HW)
    xb = np.ascontiguousarray(_to_bf16(x))
    # pack weights: per cc chunk rows [128], cols [wq(256)|wv(256)|wk(256)|wo(256)]
    wq_t = np.asarray(Wq, dtype=np.float32).T.reshape(2, 128, C)
    wv_t = np.asarray(Wv, dtype=np.float32).T.reshape(2, 128, C)
    wk_t = np.asarray(Wk, dtype=np.float32).T.reshape(2, 128, C)
    wo_t = np.asarray(Wo, dtype=np.float32).T.reshape(2, 128, C)
    wall = np.concatenate([wq_t, wv_t, wk_t, wo_t], axis=2)  # [2, 128, 1024]
    wall = np.ascontiguousarray(_to_bf16(wall))
    bo2 = np.ascontiguousarray(np.asarray(bo, dtype=np.float32).reshape(C, 1))
    return [
        {"x": xb[i * NB:(i + 1) * NB], "wall": wall, "bo": bo2}
        for i in range(NCORES)
    ]


def kernel(x, Wq, Wk, Wv, Wo, bo):
    nc = _get_nc(repeat=1)
    in_maps = make_in_maps(x, Wq, Wk, Wv, Wo, bo)
    res = bass_utils.run_bass_kernel_spmd(nc, in_maps, core_ids=list(range(NCORES)))
    out = np.concatenate([res.results[i]["out"] for i in range(NCORES)], axis=0)
    return np.ascontiguousarray(out.reshape(B, C, H, W).astype(np.float32))


# revision 10
# speedup vs baseline: 8.1354x; 2.3154x over previous
"""Trainium2 Bass kernel for ConvolutionalAttention2D (linear attention with 1x1 convs).

Reference computation (per batch b):
    q = Wq x ; k = Wk x ; v = Wv x          (1x1 convs == channel matmuls)
    phi(t) = elu(t) + 1
    qv = phi(q) @ phi(v)^T                  ([C, C] context matrix, contract over pixels)
    out = Wo (qv @ phi(k)) + bo

Kernel strategy (8 NeuronCores, data-parallel over batch B=16 -> 2 batches/core):
  - All PE traffic in bf16 (x converted on host); fp32 PSUM accumulate.
  - Algebraic refactor: Wo (qv @ phi_k) == (Wo qv) @ phi_k.
  - phi(t) = min(exp(t), 1) + relu(t), split across ACT/DVE/GPSIMD per-span
    to balance engine load (ACT does only the mandatory Exp pass by default).
  - Out-copies (PSUM->SBUF + bias) placed mostly on ACT slack.
"""

from contextlib import ExitStack

import numpy as np

import concourse.bacc as bacc
import concourse.tile as tile
from concourse import mybir
from concourse import bass_utils

B, C, H, W = 16, 256, 64, 64
HW = H * W
NCORES = 8
NB = B // NCORES  # batches per core

FP = mybir.dt.float32
BF = mybir.dt.bfloat16
F32R = mybir.dt.float32r
AF = mybir.ActivationFunctionType
OP = mybir.AluOpType

# dtype for x and the projection weights on the PE (bisect flag):
# True -> bf16 (half DMA, FWL); False -> float32r (v1-style)
XW_BF16 = True


def _register_phi_op():
    """Register a fused DVE op: out = max(min(in0, s0), in1 + s1).

    With in0 = exp(x) and s0 = s1 = 1 this is exactly phi(x) = elu(x)+1:
    for x <= 0, e^x >= 1+x so max picks e^x (and e^x <= 1); for x > 0,
    min clamps to 1 and max picks 1+x.
    """
    from concourse import dve_ops as D
    from concourse.dve_spec import Spec, Src0, Src1, C0, C1, maxx, minn, lower, _has_src1
    from concourse.dve_uop import DveOpSpec

    name = "PHI_COMBINE_ANT"
    for op in D.OPS:
        if op.name == name:
            return op
    spec = Spec(
        body=maxx(minn(Src0, C0), Src1 + C1),
        reference=lambda in0, in1, s0, s1, imm2: np.maximum(
            np.minimum(in0.astype(np.float32), s0), in1.astype(np.float32) + s1
        ),
    )
    shas = {}
    for ver in ("v3", "v4"):
        u = lower(spec, ver=ver)
        shas[ver] = DveOpSpec(
            name=name, opcode=0, uops=u, rd1_en=_has_src1(spec)
        ).sha(ver)
    op = D.DveOp(name, spec, subdim=False, uops_sha=shas)
    D.OPS.append(op)
    D._SUB_OPCODE_FOR_NAME[name] = D._CUSTOM_DVE_ROW_BASE + len(D.OPS) - 1
    D.CUSTOM_DVE_SPECS[name] = spec
    return op


PHI_OP = _register_phi_op()

# per-batch span schemes: 24 spans (16 stage-B + 8 stage-A)
#  'f': ACT Exp ; fused DVE max(min(e,1), psum+1)             [default]
#  'a': ACT Exp ; DVE min(e,1) 4x ; DVE stt(psum max 0 + t)
#  'b': ACT Exp ; ACT Relu ; DVE stt(e min 1 + r) all-bf16
DEFAULT_SPANS = "f" * 24
# per-batch out-copy engine for the 8 stage-D tiles: 'A' = ACT w/ bias, 'V' = DVE
DEFAULT_OUTS = "AAAAAAVV"


def flat2(ap):
    return ap.rearrange("p a b -> p (a b)")


def build_kernel(repeat: int = 1, span_pattern=DEFAULT_SPANS, out_pattern=DEFAULT_OUTS,
                 xp_bufs=4, phikp_bufs=2, pqvp_bufs=2, mm_bufs=3, tmps_bufs=6,
                 outp_bufs=6, out_fp32=True):
    """Build the per-core Bass program. `repeat` wraps the whole body in a
    dynamic For_i loop (used only for wall-clock timing runs)."""
    nc = bacc.Bacc("TRN2", target_bir_lowering=False, debug=False)

    XW = BF if XW_BF16 else F32R
    x_d = nc.dram_tensor("x", [NB, 2, 128, HW], XW, kind="ExternalInput")
    # weights packed per cc-chunk: [wqv(512) | wk(256) | wo(256)]
    w_d = nc.dram_tensor("wall", [2, 128, 1024], XW, kind="ExternalInput")
    bo_d = nc.dram_tensor("bo", [C, 1], FP, kind="ExternalInput")
    OD = FP if out_fp32 else BF
    out_d = nc.dram_tensor("out", [NB, C, HW], OD, kind="ExternalOutput")

    with tile.TileContext(nc) as tc, ExitStack() as ctx:
        singles = ctx.enter_context(tc.tile_pool(name="singles", bufs=1))
        xp = ctx.enter_context(tc.tile_pool(name="xp", bufs=xp_bufs))
        phikp = ctx.enter_context(tc.tile_pool(name="phikp", bufs=phikp_bufs))
        pqvp = ctx.enter_context(tc.tile_pool(name="pqvp", bufs=pqvp_bufs))
        tmps = ctx.enter_context(tc.tile_pool(name="tmps", bufs=tmps_bufs))
        smalls = ctx.enter_context(tc.tile_pool(name="smalls", bufs=2))
        outp = ctx.enter_context(tc.tile_pool(name="outp", bufs=outp_bufs))
        psmm = ctx.enter_context(tc.tile_pool(name="psmm", bufs=mm_bufs, space="PSUM"))
        psacc = ctx.enter_context(tc.tile_pool(name="psacc", bufs=1, space="PSUM"))

        # ---- weights (loaded once, replicated) ----
        w_all = singles.tile([128, 2, 1024], XW, tag="wall")
        nc.sync.dma_start(out=w_all[:], in_=w_d.ap().rearrange("cc p b -> p cc b"))
        wqv = [w_all[:, cc, 0:512] for cc in range(2)]
        wk = [w_all[:, cc, 512:768] for cc in range(2)]
        wo = [w_all[:, cc, 768:1024] for cc in range(2)]
        bo_sb = singles.tile([128, 2], FP, tag="bo")
        for m in range(2):
            nc.sync.dma_start(
                out=bo_sb[:, m:m + 1], in_=bo_d.ap()[m * 128:(m + 1) * 128, :]
            )

        state = {"span": 0, "out": 0}

        def phi_span(psum3_ap, dst_ap):
            """dst = phi(psum) = max(min(exp(x), 1), x + 1), bf16 out.

            psum3_ap must be rank-3 [128, a, b] (the custom-DVE STT struct
            requires a 2-free-dim src1); dst_ap free size must match.
            """
            i = state["span"] % 24
            state["span"] += 1
            sch = span_pattern[i]
            flat = psum3_ap.rearrange("p a b -> p (a b)")
            e = tmps.tile([128, 1024], BF, tag="e")
            nc.scalar.activation(e[:], flat, AF.Exp)
            if sch == "f":
                nc.vector._custom_dve(
                    PHI_OP, out=dst_ap, in0=e[:], in1=psum3_ap, s0=1.0, s1=1.0
                )
            elif sch == "b":
                r = tmps.tile([128, 1024], BF, tag="r")
                nc.scalar.activation(r[:], flat, AF.Relu)
                nc.vector.scalar_tensor_tensor(dst_ap, e[:], 1.0, r[:], OP.min, OP.add)
            else:
                t = tmps.tile([128, 1024], BF, tag="t")
                nc.vector.tensor_scalar_min(t[:], e[:], 1.0)
                nc.vector.scalar_tensor_tensor(dst_ap, flat, 0.0, t[:], OP.max, OP.add)

        def body(_iv=None):
            state["span"] = 0
            state["out"] = 0
            for b in range(NB):
                # ---- load x for this batch in column blocks ----
                X = [xp.tile([128, HW], XW, tag="x", name=f"x{b}_{cc}") for cc in range(2)]
                xblocks = [(0, 512), (512, 512), (1024, 1024), (2048, 1024), (3072, 1024)]
                for (c0, cw) in xblocks:
                    cs = slice(c0, c0 + cw)
                    for cc in range(2):
                        nc.sync.dma_start(
                            out=X[cc][:, cs],
                            in_=x_d.ap()[b, cc, :, cs],
                        )

                # ---- stage B: phi(q^T), phi(v^T), transposed layout [n, o] ----
                # pqv_sb[:, nchunk, 0:256] = phi_qT, [:, nchunk, 256:512] = phi_vT
                pqv_sb = pqvp.tile([128, 32, 512], BF, tag="pqv")
                for i in range(16):
                    ps = psmm.tile([128, 2, 512], FP, tag="mm")
                    for j in range(2):
                        nk = i * 2 + j
                        for cc in range(2):
                            nc.tensor.matmul(
                                ps[:, j, :],
                                X[cc][:, nk * 128:(nk + 1) * 128],
                                wqv[cc],
                                start=(cc == 0),
                                stop=(cc == 1),
                            )
                    phi_span(ps[:], pqv_sb[:, i * 2:(i + 1) * 2, :])

                # ---- stage A: phi_k = phi(Wk x), natural layout [o, n] ----
                phik = []
                for m in range(2):
                    pk = phikp.tile([128, HW], BF, tag="phik")
                    phik.append(pk)
                    for i in range(4):
                        ps = psmm.tile([128, 2, 512], FP, tag="mm")
                        for j in range(2):
                            n0 = (i * 2 + j) * 512
                            for cc in range(2):
                                nc.tensor.matmul(
                                    ps[:, j, :],
                                    wk[cc][:, m * 128:(m + 1) * 128],
                                    X[cc][:, n0:n0 + 512],
                                    start=(cc == 0),
                                    stop=(cc == 1),
                                )
                        phi_span(ps[:], pk[:, i * 1024:(i + 1) * 1024])

                # ---- stage C: qv[c, d] = sum_n phi_qT[n, c] phi_vT[n, d] ----
                # interleaved cc accumulation chains -> each in its own bank
                qv_ps = psacc.tile([128, 2, 512], FP, tag="acc")
                for i in range(32):
                    for cc in range(2):
                        nc.tensor.matmul(
                            qv_ps[:, cc, 0:256],
                            pqv_sb[:, i, cc * 128:(cc + 1) * 128],
                            pqv_sb[:, i, 256:512],
                            start=(i == 0),
                            stop=(i == 31),
                        )
                qv_sb = smalls.tile([128, 2, 256], BF, tag="qv_sb")
                nc.scalar.activation(qv_sb[:], qv_ps[:, :, 0:256], AF.Copy)

                # ---- stage C2: W2^T[d, o] = sum_c qv[c, d] WoT[c, o] ----
                w2_ps = psacc.tile([128, 2, 256], FP, tag="acc")
                for dd in range(2):
                    for cc in range(2):
                        nc.tensor.matmul(
                            w2_ps[:, dd, :],
                            qv_sb[:, cc, dd * 128:(dd + 1) * 128],
                            wo[cc],
                            start=(cc == 0),
                            stop=(cc == 1),
                        )
                w2_sb = smalls.tile([128, 2, 256], BF, tag="w2_sb")
                nc.scalar.activation(flat2(w2_sb[:]), flat2(w2_ps[:]), AF.Copy)

                # ---- stage D: out[o, n] = sum_d W2[o, d] phi_k[d, n] + bo ----
                for m in range(2):
                    for i in range(4):
                        ps = psmm.tile([128, 1024], FP, tag="mm")
                        for j in range(2):
                            n0 = (i * 2 + j) * 512
                            for dd in range(2):
                                nc.tensor.matmul(
                                    ps[:, j * 512:(j + 1) * 512],
                                    w2_sb[:, dd, m * 128:(m + 1) * 128],
                                    phik[dd][:, n0:n0 + 512],
                                    start=(dd == 0),
                                    stop=(dd == 1),
                                )
                        o_sb = outp.tile([128, 1024], OD, tag="osb")
                        oe = out_pattern[state["out"] % 8]
                        if oe == "A":
                            nc.scalar.activation(
                                o_sb[:], ps[:], AF.Identity, bias=bo_sb[:, m:m + 1]
                            )
                        else:
                            nc.vector.tensor_scalar_add(o_sb[:], ps[:], bo_sb[:, m:m + 1])
                        state["out"] += 1
                        nc.sync.dma_start(
                            out=out_d.ap()[b, m * 128:(m + 1) * 128, i * 1024:(i + 1) * 1024],
                            in_=o_sb[:],
                        )

        if repeat == 1:
            body()
        else:
            with tc.For_i(0, repeat, 1) as iv:
                body(iv)

    nc.compile()
    return nc


_nc_cache = {}


def _get_nc(repeat: int = 1):
    if repeat not in _nc_cache:
        _nc_cache[repeat] = build_kernel(repeat)
    return _nc_cache[repeat]


def _to_bf16(a):
    import ml_dtypes
    return np.asarray(a, dtype=np.float32).astype(ml_dtypes.bfloat16)


def make_in_maps(x, Wq, Wk, Wv, Wo, bo):
    x = np.asarray(x, dtype=np.float32).reshape(B, 2, 128, HW)
    xb = np.ascontiguousarray(_to_bf16(x) if XW_BF16 else x)
    # pack weights: per cc chunk rows [128], cols [wq(256)|wv(256)|wk(256)|wo(256)]
    wq_t = np.asarray(Wq, dtype=np.float32).T.reshape(2, 128, C)
    wv_t = np.asarray(Wv, dtype=np.float32).T.reshape(2, 128, C)
    wk_t = np.asarray(Wk, dtype=np.float32).T.reshape(2, 128, C)
    wo_t = np.asarray(Wo, dtype=np.float32).T.reshape(2, 128, C)
    wall = np.concatenate([wq_t, wv_t, wk_t, wo_t], axis=2)  # [2, 128, 1024]
    wall = np.ascontiguousarray(_to_bf16(wall) if XW_BF16 else wall)
    bo2 = np.ascontiguousarray(np.asarray(bo, dtype=np.float32).reshape(C, 1))
    return [
        {"x": xb[i * NB:(i + 1) * NB], "wall": wall, "bo": bo2}
        for i in range(NCORES)
    ]


def kernel(x, Wq, Wk, Wv, Wo, bo):
    nc = _get_nc(repeat=1)
    in_maps = make_in_maps(x, Wq, Wk, Wv, Wo, bo)
    res = bass_utils.run_bass_kernel_spmd(nc, in_maps, core_ids=list(range(NCORES)))
    out = np.concatenate([res.results[i]["out"] for i in range(NCORES)], axis=0)
    return np.ascontiguousarray(out.reshape(B, C, H, W).astype(np.float32))
